# revision 3
# baseline (speedup 1.0000x reference)
"""LocalSelfAttention TRN2 kernel.

Full inputs -> full output. Sharding: 8 cores = 4 batches x 2 head-groups
(8 heads each). Each core computes qkv for its heads, banded local attention
(window 128), and a partial projection (contraction over its 512 channels);
host sums the two partial projections per batch.

All matmuls run as float32r (tf32-like rounding of inputs, exact fp32
accumulate): 4x the fp32 matmul rate on the PE at ~2.4e-4 input rounding
error.

Layouts on chip (per core):
  phase 1: qkv in [t, o] orientation; x chunks [c,t] are the stationary
           operand (streamed once per q/k/v pass to bound SBUF), weights
           stream. v is kept in [t, (h,d)] layout augmented with a ones
           column per head (row 64 of the PV output becomes the softmax
           denominator).
  phase 2 (fused into 1): PE-transposes q,k head-pair columns into
           qT/kT [d, t].
  phase 3: per (head, q-pair of 256): scoresT [k_tile 128, q 256] =
           kT_chunk.T @ qT slice; triangular masks on the band edges;
           exp on ACT (scale=1/sqrt(dk)/... folded 0.125); PV accumulates
           v_aug.T @ expT into [65, 256]; row 64 = denominator ->
           reciprocal -> K=1 ones matmul broadcast -> normalize into
           attnT [c_local, t].
  phase 4: projection yT [o, t] = wp_chunk.T @ attnT, DMA out; host adds
           the two head-group partials and transposes.
"""
import sys

sys.path.insert(0, "/opt/trn_rl_repo")

from contextlib import ExitStack

import numpy as np

import concourse.bass as bass
import concourse.tile as tile
from concourse import bacc, mybir
from concourse.bass_utils import run_bass_kernel_spmd
from neuron_dtypes import static_cast_fp32_to_fp32r

F32 = mybir.dt.float32
F32R = mybir.dt.float32r
EXP = mybir.ActivationFunctionType.Exp

B, T, C = 4, 2048, 1024
NHEAD, DK, WINDOW = 16, 64, 128
HL = 8          # heads per core
NT = 16         # t tiles of 128
NCT = 8         # contraction c tiles of 128
NQP = 8         # q pairs of 256
VW = HL * 65    # v_aug row width per t tile (8 heads x (64 d + 1 ones))
SCALE = 1.0 / 8.0  # 1/sqrt(DK)

_PROGRAM = None


def round_f32r(x):
    x = np.ascontiguousarray(x, dtype=np.float32)
    return static_cast_fp32_to_fp32r(x).view(np.uint32).view(np.float32)


def build_program():
    nc = bacc.Bacc("TRN2", target_bir_lowering=False, debug=False)

    xc_d = nc.dram_tensor("xc", [NT, NCT, 128, 128], F32R, kind="ExternalInput").ap()
    wqk_d = nc.dram_tensor("wqk", [3, NCT, 128, 512], F32R, kind="ExternalInput").ap()
    wp_d = nc.dram_tensor("wp", [4, 128, 1024], F32R, kind="ExternalInput").ap()
    ident_d = nc.dram_tensor("ident", [128, 256], F32R, kind="ExternalInput").ap()
    ones_d = nc.dram_tensor("ones", [128, 8], F32R, kind="ExternalInput").ap()
    onesk_d = nc.dram_tensor("onesk", [1, 64], F32R, kind="ExternalInput").ap()
    mga_d = nc.dram_tensor("mga", [128, 128], F32, kind="ExternalInput").ap()
    mgb_d = nc.dram_tensor("mgb", [128, 128], F32, kind="ExternalInput").ap()
    yt_d = nc.dram_tensor("yt", [1024, 2048], F32, kind="ExternalOutput").ap()

    with tile.TileContext(nc) as tc, ExitStack() as ctx:
        perm = ctx.enter_context(tc.tile_pool(name="perm", bufs=1))

        ident_sb = perm.tile([128, 256], F32R, tag="ident", name="ident_sb")
        nc.sync.dma_start(ident_sb[:], ident_d[:])
        zeros_sb = ident_sb[:, 128:256]
        ones_sb = perm.tile([128, 8], F32R, tag="ones", name="ones_sb")
        nc.sync.dma_start(ones_sb[:], ones_d[:])
        onesk_sb = perm.tile([1, 64], F32R, tag="onesk", name="onesk_sb")
        nc.sync.dma_start(onesk_sb[:], onesk_d[:])
        mga_sb = perm.tile([128, 128], F32, tag="mga", name="mga_sb")
        nc.sync.dma_start(mga_sb[:], mga_d[:])
        mgb_sb = perm.tile([128, 128], F32, tag="mgb", name="mgb_sb")
        nc.sync.dma_start(mgb_sb[:], mgb_d[:])

        v_all = perm.tile([128, NT * VW], F32R, tag="vall", name="v_all")
        qT = [perm.tile([128, 2048], F32R, tag=f"qT{j}", name=f"qT{j}") for j in range(4)]
        kT = [perm.tile([128, 2048], F32R, tag=f"kT{j}", name=f"kT{j}") for j in range(4)]
        attnT = [perm.tile([128, 2048], F32R, tag=f"attnT{j}", name=f"attnT{j}")
                 for j in range(4)]

        # ---------------- phase 1: qkv + transposes ----------------
        for part in range(3):  # 0=q, 1=k, 2=v
            with tc.tile_pool(name=f"wq{part}", bufs=1) as wpool, \
                 tc.tile_pool(name=f"xp{part}", bufs=3) as xp, \
                 tc.tile_pool(name=f"qkvps{part}", bufs=3, space="PSUM") as qkvps, \
                 tc.tile_pool(name=f"qksb{part}", bufs=3) as qksb, \
                 tc.tile_pool(name=f"tpps{part}", bufs=4, space="PSUM") as tpps:
                w_sb = []
                for c in range(NCT):
                    w_c = wpool.tile([128, 512], F32R, tag=f"w{c}", name=f"w{part}_{c}")
                    nc.sync.dma_start(w_c[:], wqk_d[part, c])
                    w_sb.append(w_c)
                for t in range(NT):
                    x_t = xp.tile([128, 1024], F32R, tag="x", name=f"x{part}_{t}")
                    nc.sync.dma_start(
                        x_t[:].rearrange("p (c j) -> p c j", c=NCT),
                        xc_d[t].rearrange("c p j -> p c j"),
                    )
                    ps = qkvps.tile([128, 512], F32, tag="qkv", name=f"ps{part}_{t}")
                    for c in range(NCT):
                        nc.tensor.matmul(
                            ps[:], x_t[:, c * 128:(c + 1) * 128], w_sb[c][:],
                            start=(c == 0), stop=(c == NCT - 1),
                        )
                    if part < 2:
                        qk_t = qksb.tile([128, 512], F32R, tag="qk", name=f"qk{part}_{t}")
                        nc.any.tensor_copy(qk_t[:], ps[:])
                        dest = qT if part == 0 else kT
                        for j in range(4):
                            pst = tpps.tile([128, 128], F32R, tag="tp",
                                            name=f"tp{part}_{t}_{j}")
                            nc.tensor.transpose(
                                pst[:], qk_t[:, j * 128:(j + 1) * 128],
                                ident_sb[:, 0:128],
                            )
                            nc.any.tensor_copy(
                                dest[j][:, t * 128:(t + 1) * 128], pst[:])
                    else:
                        vs = v_all[:, t * VW:(t + 1) * VW].rearrange(
                            "p (h e) -> p h e", h=HL)
                        nc.any.tensor_copy(
                            vs[:, :, 0:64],
                            ps[:].rearrange("p (h d) -> p h d", h=HL))
                        nc.any.tensor_copy(
                            vs[:, :, 64:65],
                            ones_sb[:].rearrange("p (h o) -> p h o", o=1))

        # ---------------- phase 3: attention ----------------
        with tc.tile_pool(name="qkps", bufs=4, space="PSUM") as qkps, \
             tc.tile_pool(name="epool", bufs=6) as epool, \
             tc.tile_pool(name="tmpp", bufs=4) as tmpp, \
             tc.tile_pool(name="pvps", bufs=2, space="PSUM") as pvps, \
             tc.tile_pool(name="bcps", bufs=1, space="PSUM") as bcps, \
             tc.tile_pool(name="rcp", bufs=3) as rcp:
            for h in range(HL):
                j, ro = h // 2, (h % 2) * 64
                for qp in range(NQP):
                    kbs = [kb for kb in range(2 * qp - 1, 2 * qp + 3)
                           if 0 <= kb < NT]
                    es = []
                    for kb in kbs:
                        r = kb - 2 * qp
                        ps_s = qkps.tile([128, 256], F32, tag="s",
                                         name=f"s{h}_{qp}_{kb}")
                        nc.tensor.matmul(
                            ps_s[:],
                            kT[j][ro:ro + 64, kb * 128:(kb + 1) * 128],
                            qT[j][ro:ro + 64, qp * 256:(qp + 1) * 256],
                            start=True, stop=True,
                        )
                        e = epool.tile([128, 256], F32R, tag="e",
                                       name=f"e{h}_{qp}_{kb}")

                        def exp_direct(c0):
                            nc.scalar.activation(
                                e[:, c0:c0 + 128], ps_s[:, c0:c0 + 128],
                                EXP, scale=SCALE)

                        def exp_masked(c0, m):
                            tmp = tmpp.tile([128, 128], F32, tag="tmp",
                                            name=f"tm{h}_{qp}_{kb}")
                            nc.vector.tensor_add(tmp[:], ps_s[:, c0:c0 + 128], m[:])
                            nc.scalar.activation(
                                e[:, c0:c0 + 128], tmp[:], EXP, scale=SCALE)

                        def zero_half(c0):
                            nc.any.tensor_copy(e[:, c0:c0 + 128], zeros_sb)

                        if r == -1:
                            exp_masked(0, mga_sb)
                            zero_half(128)
                        elif r == 0:
                            exp_direct(0)
                            exp_masked(128, mga_sb)
                        elif r == 1:
                            exp_masked(0, mgb_sb)
                            exp_direct(128)
                        else:
                            zero_half(0)
                            exp_masked(128, mgb_sb)
                        es.append(e)

                    ps_o = pvps.tile([65, 256], F32, tag="o", name=f"o{h}_{qp}")
                    for i, kb in enumerate(kbs):
                        nc.tensor.matmul(
                            ps_o[:],
                            v_all[:, kb * VW + h * 65: kb * VW + (h + 1) * 65],
                            es[i][:],
                            start=(i == 0), stop=(i == len(kbs) - 1),
                        )
                    rc = rcp.tile([1, 256], F32R, tag="rc", name=f"rc{h}_{qp}")
                    with nc.allow_low_precision(reason="f32r is 12-bit-mantissa fp32; fine for softmax denom"):
                        nc.vector.reciprocal(rc[:], ps_o[64:65, :])
                    ps_b = bcps.tile([64, 256], F32, tag="b", name=f"b{h}_{qp}")
                    nc.tensor.matmul(ps_b[:], onesk_sb[:], rc[:],
                                     start=True, stop=True)
                    rb = rcp.tile([64, 256], F32, tag="rb", name=f"rb{h}_{qp}")
                    nc.scalar.copy(rb[:], ps_b[:])
                    nc.vector.tensor_mul(
                        attnT[j][ro:ro + 64, qp * 256:(qp + 1) * 256],
                        ps_o[0:64, :], rb[:])

        # ---------------- phase 4: projection ----------------
        with tc.tile_pool(name="wppool", bufs=1) as wppool, \
             tc.tile_pool(name="yps", bufs=3, space="PSUM") as yps, \
             tc.tile_pool(name="yp", bufs=3) as yp:
            wp_sb = []
            for cl in range(4):
                w_cl = wppool.tile([128, 1024], F32R, tag=f"wp{cl}", name=f"wp{cl}")
                nc.sync.dma_start(w_cl[:], wp_d[cl])
                wp_sb.append(w_cl)
            for t4 in range(4):
                for o in range(8):
                    ps_y = yps.tile([128, 512], F32, tag="y", name=f"y{t4}_{o}")
                    for cl in range(4):
                        nc.tensor.matmul(
                            ps_y[:],
                            wp_sb[cl][:, o * 128:(o + 1) * 128],
                            attnT[cl][:, t4 * 512:(t4 + 1) * 512],
                            start=(cl == 0), stop=(cl == 3),
                        )
                    y_o = yp.tile([128, 512], F32, tag="yo", name=f"yo{t4}_{o}")
                    nc.any.tensor_copy(y_o[:], ps_y[:])
                    nc.sync.dma_start(
                        yt_d[o * 128:(o + 1) * 128, t4 * 512:(t4 + 1) * 512],
                        y_o[:])

    nc.compile()
    return nc


def get_program():
    global _PROGRAM
    if _PROGRAM is None:
        _PROGRAM = build_program()
    return _PROGRAM


def _consts():
    ident = np.zeros((128, 256), dtype=np.float32)
    ident[:, 0:128] = np.eye(128, dtype=np.float32)
    i = np.arange(128)
    mga = np.where(i[:, None] >= i[None, :], 0.0, -1e9).astype(np.float32)
    mgb = np.where(i[:, None] <= i[None, :], 0.0, -1e9).astype(np.float32)
    return {
        "ident": ident,
        "ones": np.ones((128, 8), dtype=np.float32),
        "onesk": np.ones((1, 64), dtype=np.float32),
        "mga": mga,
        "mgb": mgb,
    }


def prepare_in_maps(x, w_qkv, w_proj):
    x = np.asarray(x, dtype=np.float32)
    w_qkv = np.asarray(w_qkv, dtype=np.float32)
    w_proj = np.asarray(w_proj, dtype=np.float32)
    consts = _consts()

    # per-batch x chunks: xc[t, c, i, j] = x[b].T[c*128+i, t*128+j]
    xcs = []
    for b in range(B):
        xt = np.ascontiguousarray(x[b].T)            # [1024, 2048]
        xc = xt.reshape(NCT, 128, NT, 128).transpose(2, 0, 1, 3)
        xcs.append(round_f32r(np.ascontiguousarray(xc)))

    # per head-group weights
    wqks, wps = [], []
    for hg in range(2):
        rows = []
        for part in range(3):
            sel = w_qkv[part * C + hg * 512: part * C + (hg + 1) * 512, :]  # [512,1024]
            # chunks [NCT, 128, 512]: wqk[part][c][i, o] = sel[o, c*128+i]
            rows.append(sel.T.reshape(NCT, 128, 512))
        wqks.append(round_f32r(np.ascontiguousarray(np.stack(rows, 0))))
        wp = w_proj[:, hg * 512:(hg + 1) * 512].T.reshape(4, 128, 1024)
        wps.append(round_f32r(np.ascontiguousarray(wp)))

    in_maps = []
    for b in range(B):
        for hg in range(2):
            m = {"xc": xcs[b], "wqk": wqks[hg], "wp": wps[hg]}
            m.update(consts)
            in_maps.append(m)
    return in_maps


def kernel(x, w_qkv, w_proj):
    nc = get_program()
    in_maps = prepare_in_maps(x, w_qkv, w_proj)
    res = run_bass_kernel_spmd(nc, in_maps, core_ids=list(range(8)))

    y = np.empty((B, T, C), dtype=np.float32)
    for b in range(B):
        yt = res.results[b * 2]["yt"] + res.results[b * 2 + 1]["yt"]
        y[b] = yt.T
    return y


# revision 17
# speedup vs baseline: 1.4125x; 1.4125x over previous
"""LocalSelfAttention TRN2 kernel.

Full inputs -> full output. Sharding: 8 cores = 4 batches x 2 head-groups
(8 heads each). Each core computes qkv for its heads, banded local attention
(window 128), and a partial projection (contraction over its 512 channels);
host sums the two partial projections per batch.

All matmuls run as float32r (tf32-like 12-bit-mantissa rounding of inputs,
exact fp32 accumulate): 4x the fp32 matmul rate on the PE at ~2.4e-4 input
rounding error. HW-validated: f32r matmul on pre-rounded inputs is
bit-exact vs fp32.

Per-core structure:
  phase 1 (fused): per t-tile of 128 tokens, one x chunk [c,t] DMA
    (contiguous), 24 accumulating matmuls -> q,k,v [t, o] in 3 psum banks;
    q,k PE-transposed per head-pair into qT/kT [d, t]; v copied (strided)
    into v_aug [t, (h, d|1)] with a ones column per head.
  phase 2: per (head h, q-pair of 256 tokens): banded scoresT
    [k-tile 128, q 256] = kT_chunk.T @ qT slice for the 3-4 k-tiles in the
    window; triangular/full masks added on DVE only where the band needs
    them; exp on ACT (scale 1/8) -> e tiles (f32r); PV accumulates
    v_aug.T @ e into [65, 256] whose row 64 is the softmax denominator;
    DVE reciprocal -> GPSIMD partition_broadcast -> DVE multiply writes
    normalized attnT [c_local, t].
  phase 3: projection yT[o, t] partial = wp_chunk.T @ attnT; DMA out.
Host: y[b] = (yt_hg0 + yt_hg1).T
"""
import sys

sys.path.insert(0, "/opt/trn_rl_repo")

from contextlib import ExitStack

import numpy as np

import concourse.bass as bass
import concourse.tile as tile
from concourse import bacc, mybir
from concourse.bass_utils import run_bass_kernel_spmd
from neuron_dtypes import static_cast_fp32_to_fp32r

F32 = mybir.dt.float32
F32R = mybir.dt.float32r
EXP = mybir.ActivationFunctionType.Exp

B, T, C = 4, 2048, 1024
NHEAD, DK, WINDOW = 16, 64, 128
HL = 8          # heads per core
NT = 16         # t tiles of 128
NCT = 8         # contraction c tiles of 128
NQP = 8         # q pairs of 256
VW = HL * 65    # v_aug row width per t tile (8 heads x (64 d + 1 ones))
SCALE = 1.0 / 8.0  # 1/sqrt(DK)

_PROGRAM = None


def round_f32r(x):
    x = np.ascontiguousarray(x, dtype=np.float32)
    return static_cast_fp32_to_fp32r(x).view(np.uint32).view(np.float32)


def build_program():
    nc = bacc.Bacc("TRN2", target_bir_lowering=False, debug=False)

    xc_d = nc.dram_tensor("xc", [NT, 128, 1024], F32R, kind="ExternalInput").ap()
    wqkv_d = nc.dram_tensor("wqkv", [NCT, 128, 1536], F32R, kind="ExternalInput").ap()
    wp_d = nc.dram_tensor("wp", [4, 128, 1024], F32R, kind="ExternalInput").ap()
    ident_d = nc.dram_tensor("ident", [128, 128], F32R, kind="ExternalInput").ap()
    ones_d = nc.dram_tensor("ones", [128, 8], F32R, kind="ExternalInput").ap()
    # binary {0,1} band masks per relative k-tile offset r in {-1,0,1,2}
    bins_d = nc.dram_tensor("bins", [4, 128, 256], F32, kind="ExternalInput").ap()
    yt_d = nc.dram_tensor("yt", [1024, 2048], F32, kind="ExternalOutput").ap()

    with tile.TileContext(nc) as tc, ExitStack() as ctx:
        perm = ctx.enter_context(tc.tile_pool(name="perm", bufs=1))

        ident_sb = perm.tile([128, 128], F32R, tag="ident", name="ident_sb")
        nc.sync.dma_start(ident_sb[:], ident_d[:])
        ones_sb = perm.tile([128, 8], F32R, tag="ones", name="ones_sb")
        nc.sync.dma_start(ones_sb[:], ones_d[:])
        bins_sb = perm.tile([128, 4 * 256], F32, tag="bins", name="bins_sb")
        nc.sync.dma_start(bins_sb[:].rearrange("p (r c) -> p r c", r=4),
                          bins_d[:].rearrange("r p c -> p r c"))


        v_all = perm.tile([128, NT * VW], F32R, tag="vall", name="v_all")
        qT = [perm.tile([128, 2048], F32R, tag=f"qT{j}", name=f"qT{j}") for j in range(4)]
        kT = [perm.tile([128, 2048], F32R, tag=f"kT{j}", name=f"kT{j}") for j in range(4)]

        # ---------------- phase 1: fused qkv + transposes ----------------
        with tc.tile_pool(name="wqkvp", bufs=1) as wqkvp, \
             tc.tile_pool(name="xp", bufs=2) as xp, \
             tc.tile_pool(name="qkvps", bufs=4, space="PSUM") as qkvps, \
             tc.tile_pool(name="qksb", bufs=2) as qksb, \
             tc.tile_pool(name="tpps", bufs=3, space="PSUM") as tpps:
            w_sb = []
            for c in range(NCT):
                w_c = wqkvp.tile([128, 1536], F32R, tag=f"w{c}", name=f"w{c}")
                nc.sync.dma_start(w_c[:], wqkv_d[c])
                w_sb.append(w_c)
            for t in range(NT):
                x_t = xp.tile([128, 1024], F32R, tag="x", name=f"x{t}")
                nc.sync.dma_start(x_t[:], xc_d[t])
                ps_q = qkvps.tile([128, 512], F32, tag="qkv", name=f"psq{t}")
                ps_k = qkvps.tile([128, 512], F32, tag="qkv", name=f"psk{t}")
                ps_v = qkvps.tile([128, 512], F32, tag="qkv", name=f"psv{t}")
                for c in range(NCT):
                    xc_c = x_t[:, c * 128:(c + 1) * 128]
                    st, sp = (c == 0), (c == NCT - 1)
                    nc.tensor.matmul(ps_q[:], xc_c, w_sb[c][:, 0:512],
                                     start=st, stop=sp)
                    nc.tensor.matmul(ps_k[:], xc_c, w_sb[c][:, 512:1024],
                                     start=st, stop=sp)
                    nc.tensor.matmul(ps_v[:], xc_c, w_sb[c][:, 1024:1536],
                                     start=st, stop=sp)
                qk_t = qksb.tile([128, 1024], F32R, tag="qk", name=f"qk{t}")
                nc.vector.tensor_copy(qk_t[:, 0:512], ps_q[:])
                nc.vector.tensor_copy(qk_t[:, 512:1024], ps_k[:])
                for j in range(4):
                    pq = tpps.tile([128, 128], F32R, tag="tp", name=f"tq{t}_{j}")
                    nc.tensor.transpose(pq[:], qk_t[:, j * 128:(j + 1) * 128],
                                        ident_sb[:])
                    nc.vector.tensor_copy(qT[j][:, t * 128:(t + 1) * 128], pq[:])
                    pk = tpps.tile([128, 128], F32R, tag="tp", name=f"tk{t}_{j}")
                    nc.tensor.transpose(pk[:], qk_t[:, 512 + j * 128:512 + (j + 1) * 128],
                                        ident_sb[:])
                    nc.vector.tensor_copy(kT[j][:, t * 128:(t + 1) * 128], pk[:])
                vs = v_all[:, t * VW:(t + 1) * VW].rearrange("p (h e) -> p h e", h=HL)
                nc.vector.tensor_copy(vs[:, :, 0:64],
                                      ps_v[:].rearrange("p (h d) -> p h d", h=HL))
                nc.vector.tensor_copy(vs[:, :, 64:65],
                                      ones_sb[:].rearrange("p (h o) -> p h o", o=1))

        attnT = [perm.tile([128, 2048], F32R, tag=f"attnT{j}", name=f"attnT{j}")
                 for j in range(4)]

        # ---------------- phase 2: attention ----------------
        # Inner loop has no reductions off the PE critical path: per k-tile
        # one matmul -> one full-width exp (ACT) -> one binary-mask multiply
        # (DVE, also the f32->f32r cast). PV output row 64 (denominator) is
        # stashed into dall; normalization is deferred and batched below.
        with tc.tile_pool(name="qkps", bufs=5, space="PSUM") as qkps, \
             tc.tile_pool(name="expool", bufs=6) as expool, \
             tc.tile_pool(name="epool", bufs=8) as epool, \
             tc.tile_pool(name="rcp", bufs=4) as rcp, \
             tc.tile_pool(name="pvps", bufs=3, space="PSUM") as pvps:
            for h in range(HL):
                j, ro = h // 2, (h % 2) * 64
                for qp in range(NQP):
                    kbs = [kb for kb in range(2 * qp - 1, 2 * qp + 3)
                           if 0 <= kb < NT]
                    es = []
                    for kb in kbs:
                        r = kb - 2 * qp
                        ps_s = qkps.tile([128, 256], F32, tag="s",
                                         name=f"s{h}_{qp}_{kb}")
                        nc.tensor.matmul(
                            ps_s[:],
                            kT[j][ro:ro + 64, kb * 128:(kb + 1) * 128],
                            qT[j][ro:ro + 64, qp * 256:(qp + 1) * 256],
                            start=True, stop=True,
                        )
                        ex = expool.tile([128, 256], F32, tag="ex",
                                         name=f"ex{h}_{qp}_{kb}")
                        nc.scalar.activation(ex[:], ps_s[:], EXP, scale=SCALE)
                        e = epool.tile([128, 256], F32R, tag="e",
                                       name=f"e{h}_{qp}_{kb}")
                        bsl = bins_sb[:, (r + 1) * 256:(r + 2) * 256]
                        nc.vector.tensor_mul(e[:], ex[:], bsl)
                        es.append(e)

                    ps_o = pvps.tile([65, 256], F32, tag="o", name=f"o{h}_{qp}")
                    for i, kb in enumerate(kbs):
                        nc.tensor.matmul(
                            ps_o[:],
                            v_all[:, kb * VW + h * 65: kb * VW + (h + 1) * 65],
                            es[i][:],
                            start=(i == 0), stop=(i == len(kbs) - 1),
                        )
                    # normalize: fast reciprocal of denominator row, GPSIMD
                    # broadcast to 64 partitions, one DVE multiply psum->sbuf
                    d_sb = rcp.tile([1, 256], F32, tag="d", name=f"d{h}_{qp}")
                    nc.scalar.copy(d_sb[:], ps_o[64:65, :])
                    rc = rcp.tile([1, 256], F32, tag="rc", name=f"rc{h}_{qp}")
                    nc.vector.reciprocal_approx_fast(rc[:], d_sb[:])
                    rb = rcp.tile([64, 256], F32, tag="rb", name=f"rb{h}_{qp}")
                    nc.gpsimd.partition_broadcast(rb[:], rc[:])
                    nc.vector.tensor_mul(
                        attnT[j][ro:ro + 64, qp * 256:(qp + 1) * 256],
                        ps_o[0:64, :], rb[:])

        # ---------------- phase 3: projection ----------------
        with tc.tile_pool(name="wppool", bufs=1) as wppool, \
             tc.tile_pool(name="yps", bufs=3, space="PSUM") as yps, \
             tc.tile_pool(name="yp", bufs=3) as yp:
            wp_sb = []
            for cl in range(4):
                w_cl = wppool.tile([128, 1024], F32R, tag=f"wp{cl}", name=f"wp{cl}")
                nc.sync.dma_start(w_cl[:], wp_d[cl])
                wp_sb.append(w_cl)
            for t4 in range(4):
                for o in range(8):
                    ps_y = yps.tile([128, 512], F32, tag="y", name=f"y{t4}_{o}")
                    for cl in range(4):
                        nc.tensor.matmul(
                            ps_y[:],
                            wp_sb[cl][:, o * 128:(o + 1) * 128],
                            attnT[cl][:, t4 * 512:(t4 + 1) * 512],
                            start=(cl == 0), stop=(cl == 3),
                        )
                    y_o = yp.tile([128, 512], F32, tag="yo", name=f"yo{t4}_{o}")
                    nc.vector.tensor_copy(y_o[:], ps_y[:])
                    nc.sync.dma_start(
                        yt_d[o * 128:(o + 1) * 128, t4 * 512:(t4 + 1) * 512],
                        y_o[:])

    nc.compile()
    return nc


def get_program():
    global _PROGRAM
    if _PROGRAM is None:
        _PROGRAM = build_program()
    return _PROGRAM


def _consts():
    # binary allow-masks in scoresT layout [k row i, q col]: for relative
    # k-tile offset r, allowed iff -128 <= r*128 + i - col <= 128
    i = np.arange(128)[:, None]
    col = np.arange(256)[None, :]
    bins = np.empty((4, 128, 256), dtype=np.float32)
    for ri, r in enumerate((-1, 0, 1, 2)):
        diff = r * 128 + i - col
        bins[ri] = ((diff >= -128) & (diff <= 128)).astype(np.float32)
    return {
        "ident": np.eye(128, dtype=np.float32),
        "ones": np.ones((128, 8), dtype=np.float32),
        "bins": bins,
    }


def prepare_in_maps(x, w_qkv, w_proj):
    x = np.asarray(x, dtype=np.float32)
    w_qkv = np.asarray(w_qkv, dtype=np.float32)
    w_proj = np.asarray(w_proj, dtype=np.float32)
    consts = _consts()

    # xc[t, p, c*128+j] = x[b, t*128+j, c*128+p]  (contiguous 4KB partition rows)
    xcs = []
    for b in range(B):
        xc = x[b].reshape(NT, 128, NCT, 128).transpose(0, 3, 2, 1)
        xcs.append(round_f32r(np.ascontiguousarray(xc).reshape(NT, 128, 1024)))

    wqkvs, wps = [], []
    for hg in range(2):
        cols = []
        for part in range(3):
            sel = w_qkv[part * C + hg * 512: part * C + (hg + 1) * 512, :]  # [512,1024]
            cols.append(sel.T)                                   # [1024, 512]
        big = np.concatenate(cols, axis=1)                       # [1024, 1536]
        wqkvs.append(round_f32r(np.ascontiguousarray(big.reshape(NCT, 128, 1536))))
        wp = w_proj[:, hg * 512:(hg + 1) * 512].T.reshape(4, 128, 1024)
        wps.append(round_f32r(np.ascontiguousarray(wp)))

    in_maps = []
    for b in range(B):
        for hg in range(2):
            m = {"xc": xcs[b], "wqkv": wqkvs[hg], "wp": wps[hg]}
            m.update(consts)
            in_maps.append(m)
    return in_maps


def kernel(x, w_qkv, w_proj):
    nc = get_program()
    in_maps = prepare_in_maps(x, w_qkv, w_proj)
    res = run_bass_kernel_spmd(nc, in_maps, core_ids=list(range(8)))

    y = np.empty((B, T, C), dtype=np.float32)
    for b in range(B):
        yt = res.results[b * 2]["yt"] + res.results[b * 2 + 1]["yt"]
        y[b] = yt.T
    return y


# revision 21
# speedup vs baseline: 1.6113x; 1.1407x over previous
"""LocalSelfAttention TRN2 kernel.

Full inputs -> full output. Sharding: 8 cores = 4 batches x 2 head-groups
(8 heads each). Each core computes qkv for its heads, banded local attention
(window 128), and a partial projection (contraction over its 512 channels);
host sums the two partial projections per batch.

All matmuls run as float32r (tf32-like 12-bit-mantissa rounding of inputs,
exact fp32 accumulate): 4x the fp32 matmul rate on the PE at ~2.4e-4 input
rounding error. HW-validated: f32r matmul on pre-rounded inputs is
bit-exact vs fp32.

Per-core structure:
  phase 1 (fused): per t-tile of 128 tokens, one x chunk [c,t] DMA
    (contiguous), 24 accumulating matmuls -> q,k,v [t, o] in 3 psum banks;
    q,k PE-transposed per head-pair into qT/kT [d, t]; v copied (strided)
    into v_aug [t, (h, d|1)] with a ones column per head.
  phase 2: per (head h, q-pair of 256 tokens): banded scoresT
    [k-tile 128, q 256] = kT_chunk.T @ qT slice for the 3-4 k-tiles in the
    window; triangular/full masks added on DVE only where the band needs
    them; exp on ACT (scale 1/8) -> e tiles (f32r); PV accumulates
    v_aug.T @ e into [65, 256] whose row 64 is the softmax denominator;
    DVE reciprocal -> GPSIMD partition_broadcast -> DVE multiply writes
    normalized attnT [c_local, t].
  phase 3: projection yT[o, t] partial = wp_chunk.T @ attnT; DMA out.
Host: y[b] = (yt_hg0 + yt_hg1).T
"""
import sys

sys.path.insert(0, "/opt/trn_rl_repo")

from contextlib import ExitStack

import numpy as np

import concourse.bass as bass
import concourse.tile as tile
from concourse import bacc, mybir
from concourse.bass_utils import run_bass_kernel_spmd
from neuron_dtypes import static_cast_fp32_to_fp32r

F32 = mybir.dt.float32
F32R = mybir.dt.float32r
EXP = mybir.ActivationFunctionType.Exp

B, T, C = 4, 2048, 1024
NHEAD, DK, WINDOW = 16, 64, 128
HL = 8          # heads per core
NT = 16         # t tiles of 128
NCT = 8         # contraction c tiles of 128
NQP = 8         # q pairs of 256
VW = HL * 65    # v_aug row width per t tile (8 heads x (64 d + 1 ones))
SCALE = 1.0 / 8.0  # 1/sqrt(DK)

_PROGRAM = None


def round_f32r(x):
    x = np.ascontiguousarray(x, dtype=np.float32)
    return static_cast_fp32_to_fp32r(x).view(np.uint32).view(np.float32)


def build_program():
    nc = bacc.Bacc("TRN2", target_bir_lowering=False, debug=False)

    xc_d = nc.dram_tensor("xc", [NT, 128, 1024], F32R, kind="ExternalInput").ap()
    wqkv_d = nc.dram_tensor("wqkv", [NCT, 128, 1536], F32R, kind="ExternalInput").ap()
    wp_d = nc.dram_tensor("wp", [4, 128, 1024], F32R, kind="ExternalInput").ap()
    ident_d = nc.dram_tensor("ident", [128, 128], F32R, kind="ExternalInput").ap()
    ones_d = nc.dram_tensor("ones", [128, 8], F32R, kind="ExternalInput").ap()
    # binary {0,1} band masks: band384 (allowed i <= c <= i+256) and
    # band256 (allowed |i - c| <= 128), concatenated [128, 384+256]
    bins_d = nc.dram_tensor("bins", [128, 640], F32, kind="ExternalInput").ap()
    zeros_d = nc.dram_tensor("zeros", [128, 128], F32R, kind="ExternalInput").ap()
    yt_d = nc.dram_tensor("yt", [1024, 2048], F32, kind="ExternalOutput").ap()

    with tile.TileContext(nc) as tc, ExitStack() as ctx:
        perm = ctx.enter_context(tc.tile_pool(name="perm", bufs=1))

        ident_sb = perm.tile([128, 128], F32R, tag="ident", name="ident_sb")
        nc.sync.dma_start(ident_sb[:], ident_d[:])
        ones_sb = perm.tile([128, 8], F32R, tag="ones", name="ones_sb")
        nc.sync.dma_start(ones_sb[:], ones_d[:])
        bins_sb = perm.tile([128, 640], F32, tag="bins", name="bins_sb")
        nc.sync.dma_start(bins_sb[:], bins_d[:])
        zeros_sb = perm.tile([128, 128], F32R, tag="zeros", name="zeros_sb")
        nc.sync.dma_start(zeros_sb[:], zeros_d[:])


        v_all = perm.tile([128, NT * VW], F32R, tag="vall", name="v_all")
        qT = [perm.tile([128, 2048], F32R, tag=f"qT{j}", name=f"qT{j}") for j in range(4)]
        kT = [perm.tile([128, 2048], F32R, tag=f"kT{j}", name=f"kT{j}") for j in range(4)]

        # ---------------- phase 1: fused qkv + transposes ----------------
        with tc.tile_pool(name="wqkvp", bufs=1) as wqkvp, \
             tc.tile_pool(name="xp", bufs=2) as xp, \
             tc.tile_pool(name="qkvps", bufs=4, space="PSUM") as qkvps, \
             tc.tile_pool(name="qksb", bufs=2) as qksb, \
             tc.tile_pool(name="tpps", bufs=3, space="PSUM") as tpps:
            w_sb = []
            for c in range(NCT):
                w_c = wqkvp.tile([128, 1536], F32R, tag=f"w{c}", name=f"w{c}")
                nc.sync.dma_start(w_c[:], wqkv_d[c])
                w_sb.append(w_c)
            for t in range(NT):
                x_t = xp.tile([128, 1024], F32R, tag="x", name=f"x{t}")
                nc.sync.dma_start(x_t[:], xc_d[t])
                ps_q = qkvps.tile([128, 512], F32, tag="qkv", name=f"psq{t}")
                ps_k = qkvps.tile([128, 512], F32, tag="qkv", name=f"psk{t}")
                ps_v = qkvps.tile([128, 512], F32, tag="qkv", name=f"psv{t}")
                for c in range(NCT):
                    xc_c = x_t[:, c * 128:(c + 1) * 128]
                    st, sp = (c == 0), (c == NCT - 1)
                    nc.tensor.matmul(ps_q[:], xc_c, w_sb[c][:, 0:512],
                                     start=st, stop=sp)
                    nc.tensor.matmul(ps_k[:], xc_c, w_sb[c][:, 512:1024],
                                     start=st, stop=sp)
                    nc.tensor.matmul(ps_v[:], xc_c, w_sb[c][:, 1024:1536],
                                     start=st, stop=sp)
                qk_t = qksb.tile([128, 1024], F32R, tag="qk", name=f"qk{t}")
                nc.vector.tensor_copy(qk_t[:, 0:512], ps_q[:])
                nc.vector.tensor_copy(qk_t[:, 512:1024], ps_k[:])
                for j in range(4):
                    pq = tpps.tile([128, 128], F32R, tag="tp", name=f"tq{t}_{j}")
                    nc.tensor.transpose(pq[:], qk_t[:, j * 128:(j + 1) * 128],
                                        ident_sb[:])
                    nc.vector.tensor_copy(qT[j][:, t * 128:(t + 1) * 128], pq[:])
                    pk = tpps.tile([128, 128], F32R, tag="tp", name=f"tk{t}_{j}")
                    nc.tensor.transpose(pk[:], qk_t[:, 512 + j * 128:512 + (j + 1) * 128],
                                        ident_sb[:])
                    nc.vector.tensor_copy(kT[j][:, t * 128:(t + 1) * 128], pk[:])
                vs = v_all[:, t * VW:(t + 1) * VW].rearrange("p (h e) -> p h e", h=HL)
                nc.vector.tensor_copy(vs[:, :, 0:64],
                                      ps_v[:].rearrange("p (h d) -> p h d", h=HL))
                nc.vector.tensor_copy(vs[:, :, 64:65],
                                      ones_sb[:].rearrange("p (h o) -> p h o", o=1))

        attnT = [perm.tile([128, 2048], F32R, tag=f"attnT{j}", name=f"attnT{j}")
                 for j in range(4)]

        # ---------------- phase 2: attention ----------------
        # Inner loop has no reductions off the PE critical path: per k-tile
        # one matmul -> one full-width exp (ACT) -> one binary-mask multiply
        # (DVE, also the f32->f32r cast). PV output row 64 (denominator) is
        # stashed into dall; normalization is deferred and batched below.
        # QK is k-tile-major: one N=512 matmul per (h, kb) covering both
        # q-pairs whose window contains kb (halves LDWEIGHTS count -> PE
        # array duty stays above the HAM warm threshold). The e tile of a
        # k-tile is shared by the two q-pairs that read 256-col slices.
        with tc.tile_pool(name="qkps", bufs=4, space="PSUM") as qkps, \
             tc.tile_pool(name="expool", bufs=5) as expool, \
             tc.tile_pool(name="epool", bufs=10) as epool, \
             tc.tile_pool(name="rcp", bufs=4) as rcp, \
             tc.tile_pool(name="pvps", bufs=3, space="PSUM") as pvps:
            for h in range(HL):
                j, ro = h // 2, (h % 2) * 64
                e_tiles = {}
                for kb in range(NT):
                    qps = [q for q in range(NQP) if 2 * q - 1 <= kb <= 2 * q + 2]
                    q0 = qps[0] * 256
                    width = 256 * len(qps)
                    ps_s = qkps.tile([128, width], F32, tag="s",
                                     name=f"s{h}_{kb}")
                    nc.tensor.matmul(
                        ps_s[:],
                        kT[j][ro:ro + 64, kb * 128:(kb + 1) * 128],
                        qT[j][ro:ro + 64, q0:q0 + width],
                        start=True, stop=True,
                    )
                    # in-band columns: |kb*128 + i - q0 - col| <= 128
                    lo = max(0, ((kb * 128 - q0 - 128) // 128) * 128)
                    hi = min(width, ((kb * 128 - q0 + 256 + 127) // 128) * 128)
                    bw = hi - lo
                    boff = kb * 128 - q0 - lo  # 0 -> band256, 128 -> band384
                    bsl = (bins_sb[:, 384:384 + bw] if boff == 0
                           else bins_sb[:, 0:bw])
                    ex = expool.tile([128, width], F32, tag="ex",
                                     name=f"ex{h}_{kb}")
                    nc.scalar.activation(ex[:, lo:hi], ps_s[:, lo:hi],
                                         EXP, scale=SCALE)
                    e = epool.tile([128, width], F32R, tag="e",
                                   name=f"e{h}_{kb}")
                    nc.vector.tensor_mul(e[:, lo:hi], ex[:, lo:hi], bsl)
                    if lo > 0:
                        nc.gpsimd.tensor_copy(e[:, 0:lo], zeros_sb[:, 0:lo])
                    if hi < width:
                        nc.gpsimd.tensor_copy(e[:, hi:width],
                                              zeros_sb[:, 0:width - hi])
                    e_tiles[kb] = (e, q0)

                for qp in range(NQP):
                    kbs = [kb for kb in range(2 * qp - 1, 2 * qp + 3)
                           if 0 <= kb < NT]
                    ps_o = pvps.tile([65, 256], F32, tag="o", name=f"o{h}_{qp}")
                    for i, kb in enumerate(kbs):
                        e, q0_e = e_tiles[kb]
                        off = qp * 256 - q0_e
                        nc.tensor.matmul(
                            ps_o[:],
                            v_all[:, kb * VW + h * 65: kb * VW + (h + 1) * 65],
                            e[:, off:off + 256],
                            start=(i == 0), stop=(i == len(kbs) - 1),
                        )
                    # normalize: fast reciprocal of denominator row, GPSIMD
                    # broadcast to 64 partitions, one DVE multiply psum->sbuf
                    d_sb = rcp.tile([1, 256], F32, tag="d", name=f"d{h}_{qp}")
                    nc.scalar.copy(d_sb[:], ps_o[64:65, :])
                    rc = rcp.tile([1, 256], F32, tag="rc", name=f"rc{h}_{qp}")
                    nc.vector.reciprocal_approx_fast(rc[:], d_sb[:])
                    rb = rcp.tile([64, 256], F32, tag="rb", name=f"rb{h}_{qp}")
                    nc.gpsimd.partition_broadcast(rb[:], rc[:])
                    nc.vector.tensor_mul(
                        attnT[j][ro:ro + 64, qp * 256:(qp + 1) * 256],
                        ps_o[0:64, :], rb[:])

        # ---------------- phase 3: projection ----------------
        with tc.tile_pool(name="wppool", bufs=1) as wppool, \
             tc.tile_pool(name="yps", bufs=3, space="PSUM") as yps, \
             tc.tile_pool(name="yp", bufs=3) as yp:
            wp_sb = []
            for cl in range(4):
                w_cl = wppool.tile([128, 1024], F32R, tag=f"wp{cl}", name=f"wp{cl}")
                nc.sync.dma_start(w_cl[:], wp_d[cl])
                wp_sb.append(w_cl)
            for t4 in range(4):
                for o in range(8):
                    ps_y = yps.tile([128, 512], F32, tag="y", name=f"y{t4}_{o}")
                    for cl in range(4):
                        nc.tensor.matmul(
                            ps_y[:],
                            wp_sb[cl][:, o * 128:(o + 1) * 128],
                            attnT[cl][:, t4 * 512:(t4 + 1) * 512],
                            start=(cl == 0), stop=(cl == 3),
                        )
                    y_o = yp.tile([128, 512], F32, tag="yo", name=f"yo{t4}_{o}")
                    nc.vector.tensor_copy(y_o[:], ps_y[:])
                    nc.sync.dma_start(
                        yt_d[o * 128:(o + 1) * 128, t4 * 512:(t4 + 1) * 512],
                        y_o[:])

    nc.compile()
    return nc


def get_program():
    global _PROGRAM
    if _PROGRAM is None:
        _PROGRAM = build_program()
    return _PROGRAM


def _consts():
    # binary allow-masks in scoresT layout [k row i, q col]:
    # band384[i, c] = 1 iff |128 + i - c| <= 128 (i.e. i <= c <= i + 256)
    # band256[i, c] = 1 iff |i - c| <= 128
    i = np.arange(128)[:, None]
    c384 = np.arange(384)[None, :]
    c256 = np.arange(256)[None, :]
    band384 = (np.abs(128 + i - c384) <= 128).astype(np.float32)
    band256 = (np.abs(i - c256) <= 128).astype(np.float32)
    return {
        "ident": np.eye(128, dtype=np.float32),
        "ones": np.ones((128, 8), dtype=np.float32),
        "bins": np.concatenate([band384, band256], axis=1),
        "zeros": np.zeros((128, 128), dtype=np.float32),
    }


def prepare_in_maps(x, w_qkv, w_proj):
    x = np.asarray(x, dtype=np.float32)
    w_qkv = np.asarray(w_qkv, dtype=np.float32)
    w_proj = np.asarray(w_proj, dtype=np.float32)
    consts = _consts()

    # xc[t, p, c*128+j] = x[b, t*128+j, c*128+p]  (contiguous 4KB partition rows)
    xcs = []
    for b in range(B):
        xc = x[b].reshape(NT, 128, NCT, 128).transpose(0, 3, 2, 1)
        xcs.append(round_f32r(np.ascontiguousarray(xc).reshape(NT, 128, 1024)))

    wqkvs, wps = [], []
    for hg in range(2):
        cols = []
        for part in range(3):
            sel = w_qkv[part * C + hg * 512: part * C + (hg + 1) * 512, :]  # [512,1024]
            cols.append(sel.T)                                   # [1024, 512]
        big = np.concatenate(cols, axis=1)                       # [1024, 1536]
        wqkvs.append(round_f32r(np.ascontiguousarray(big.reshape(NCT, 128, 1536))))
        wp = w_proj[:, hg * 512:(hg + 1) * 512].T.reshape(4, 128, 1024)
        wps.append(round_f32r(np.ascontiguousarray(wp)))

    in_maps = []
    for b in range(B):
        for hg in range(2):
            m = {"xc": xcs[b], "wqkv": wqkvs[hg], "wp": wps[hg]}
            m.update(consts)
            in_maps.append(m)
    return in_maps


def kernel(x, w_qkv, w_proj):
    nc = get_program()
    in_maps = prepare_in_maps(x, w_qkv, w_proj)
    res = run_bass_kernel_spmd(nc, in_maps, core_ids=list(range(8)))

    y = np.empty((B, T, C), dtype=np.float32)
    for b in range(B):
        yt = res.results[b * 2]["yt"] + res.results[b * 2 + 1]["yt"]
        y[b] = yt.T
    return y


# revision 27
# speedup vs baseline: 1.7113x; 1.0621x over previous
"""LocalSelfAttention TRN2 kernel.

Full inputs -> full output. Sharding: 8 cores = 4 batches x 2 head-groups
(8 heads each). Each core computes qkv for its heads, banded local attention
(window 128), and a partial projection (contraction over its 512 channels);
host sums the two partial projections per batch.

All matmuls run as float32r (tf32-like 12-bit-mantissa rounding of inputs,
exact fp32 accumulate): 4x the fp32 matmul rate on the PE at ~2.4e-4 input
rounding error. HW-validated: f32r matmul on pre-rounded inputs is
bit-exact vs fp32.

Per-core structure:
  phase 1 (fused): per t-tile of 128 tokens, one x chunk [c,t] DMA
    (contiguous), 24 accumulating matmuls -> q,k,v [t, o] in 3 psum banks;
    q,k PE-transposed per head-pair into qT/kT [d, t]; v copied (strided)
    into v_aug [t, (h, d|1)] with a ones column per head.
  phase 2: per (head h, q-pair of 256 tokens): banded scoresT
    [k-tile 128, q 256] = kT_chunk.T @ qT slice for the 3-4 k-tiles in the
    window; triangular/full masks added on DVE only where the band needs
    them; exp on ACT (scale 1/8) -> e tiles (f32r); PV accumulates
    v_aug.T @ e into [65, 256] whose row 64 is the softmax denominator;
    DVE reciprocal -> GPSIMD partition_broadcast -> DVE multiply writes
    normalized attnT [c_local, t].
  phase 3: projection yT[o, t] partial = wp_chunk.T @ attnT; DMA out.
Host: y[b] = (yt_hg0 + yt_hg1).T
"""
import sys

sys.path.insert(0, "/opt/trn_rl_repo")

from contextlib import ExitStack

import numpy as np

import concourse.bass as bass
import concourse.tile as tile
from concourse import bacc, mybir
from concourse.bass_utils import run_bass_kernel_spmd
from neuron_dtypes import static_cast_fp32_to_fp32r

F32 = mybir.dt.float32
F32R = mybir.dt.float32r
EXP = mybir.ActivationFunctionType.Exp

B, T, C = 4, 2048, 1024
NHEAD, DK, WINDOW = 16, 64, 128
HL = 8          # heads per core
NT = 16         # t tiles of 128
NCT = 8         # contraction c tiles of 128
NQP = 8         # q pairs of 256
VW = HL * 65    # v_aug row width per t tile (8 heads x (64 d + 1 ones))
SCALE = 1.0 / 8.0  # 1/sqrt(DK)

_PROGRAM = None


def round_f32r(x):
    x = np.ascontiguousarray(x, dtype=np.float32)
    return static_cast_fp32_to_fp32r(x).view(np.uint32).view(np.float32)


def build_program():
    nc = bacc.Bacc("TRN2", target_bir_lowering=False, debug=False)

    xc_d = nc.dram_tensor("xc", [NT, 128, 1024], F32R, kind="ExternalInput").ap()
    wqkv_d = nc.dram_tensor("wqkv", [NCT, 128, 1536], F32R, kind="ExternalInput").ap()
    wp_d = nc.dram_tensor("wp", [4, 128, 1024], F32R, kind="ExternalInput").ap()
    ident_d = nc.dram_tensor("ident", [128, 128], F32R, kind="ExternalInput").ap()
    ones_d = nc.dram_tensor("ones", [128, 8], F32R, kind="ExternalInput").ap()
    # binary {0,1} band masks: band384 (allowed i <= c <= i+256) and
    # band256 (allowed |i - c| <= 128), concatenated [128, 384+256]
    bins_d = nc.dram_tensor("bins", [128, 640], F32, kind="ExternalInput").ap()
    yt_d = nc.dram_tensor("yt", [1024, 2048], F32, kind="ExternalOutput").ap()

    with tile.TileContext(nc) as tc, ExitStack() as ctx:
        perm = ctx.enter_context(tc.tile_pool(name="perm", bufs=1))

        ident_sb = perm.tile([128, 128], F32R, tag="ident", name="ident_sb")
        nc.sync.dma_start(ident_sb[:], ident_d[:])
        ones_sb = perm.tile([128, 8], F32R, tag="ones", name="ones_sb")
        nc.sync.dma_start(ones_sb[:], ones_d[:])
        bins_sb = perm.tile([128, 640], F32, tag="bins", name="bins_sb")
        nc.sync.dma_start(bins_sb[:], bins_d[:])


        v_all = perm.tile([128, NT * VW], F32R, tag="vall", name="v_all")
        qT = [perm.tile([128, 2048], F32R, tag=f"qT{j}", name=f"qT{j}") for j in range(4)]
        kT = [perm.tile([128, 2048], F32R, tag=f"kT{j}", name=f"kT{j}") for j in range(4)]

        # ---------------- phase 1: fused qkv + transposes ----------------
        with tc.tile_pool(name="wqkvp", bufs=1) as wqkvp, \
             tc.tile_pool(name="xp", bufs=2) as xp, \
             tc.tile_pool(name="qkvps", bufs=4, space="PSUM") as qkvps, \
             tc.tile_pool(name="qksb", bufs=2) as qksb, \
             tc.tile_pool(name="tpps", bufs=3, space="PSUM") as tpps:
            w_sb = []
            for c in range(NCT):
                w_c = wqkvp.tile([128, 1536], F32R, tag=f"w{c}", name=f"w{c}")
                nc.sync.dma_start(w_c[:], wqkv_d[c])
                w_sb.append(w_c)
            for t in range(NT):
                x_t = xp.tile([128, 1024], F32R, tag="x", name=f"x{t}")
                nc.sync.dma_start(x_t[:], xc_d[t])
                ps_q = qkvps.tile([128, 512], F32, tag="qkv", name=f"psq{t}")
                ps_k = qkvps.tile([128, 512], F32, tag="qkv", name=f"psk{t}")
                ps_v = qkvps.tile([128, 512], F32, tag="qkv", name=f"psv{t}")
                for c in range(NCT):
                    xc_c = x_t[:, c * 128:(c + 1) * 128]
                    st, sp = (c == 0), (c == NCT - 1)
                    nc.tensor.matmul(ps_q[:], xc_c, w_sb[c][:, 0:512],
                                     start=st, stop=sp)
                    nc.tensor.matmul(ps_k[:], xc_c, w_sb[c][:, 512:1024],
                                     start=st, stop=sp)
                    nc.tensor.matmul(ps_v[:], xc_c, w_sb[c][:, 1024:1536],
                                     start=st, stop=sp)
                qk_t = qksb.tile([128, 1024], F32R, tag="qk", name=f"qk{t}")
                nc.vector.tensor_copy(qk_t[:, 0:512], ps_q[:])
                nc.vector.tensor_copy(qk_t[:, 512:1024], ps_k[:])
                for j in range(4):
                    pq = tpps.tile([128, 128], F32R, tag="tp", name=f"tq{t}_{j}")
                    nc.tensor.transpose(pq[:], qk_t[:, j * 128:(j + 1) * 128],
                                        ident_sb[:])
                    nc.vector.tensor_copy(qT[j][:, t * 128:(t + 1) * 128], pq[:])
                    pk = tpps.tile([128, 128], F32R, tag="tp", name=f"tk{t}_{j}")
                    nc.tensor.transpose(pk[:], qk_t[:, 512 + j * 128:512 + (j + 1) * 128],
                                        ident_sb[:])
                    nc.vector.tensor_copy(kT[j][:, t * 128:(t + 1) * 128], pk[:])
                vs = v_all[:, t * VW:(t + 1) * VW].rearrange("p (h e) -> p h e", h=HL)
                nc.vector.tensor_copy(vs[:, :, 0:64],
                                      ps_v[:].rearrange("p (h d) -> p h d", h=HL))
                nc.vector.tensor_copy(vs[:, :, 64:65],
                                      ones_sb[:].rearrange("p (h o) -> p h o", o=1))

        attnT = [perm.tile([128, 2048], F32R, tag=f"attnT{j}", name=f"attnT{j}")
                 for j in range(4)]

        # ---------------- phase 2: attention ----------------
        # Inner loop has no reductions off the PE critical path: per k-tile
        # one matmul -> one full-width exp (ACT) -> one binary-mask multiply
        # (DVE, also the f32->f32r cast). PV output row 64 (denominator) is
        # stashed into dall; normalization is deferred and batched below.
        # QK is k-tile-major: one N=512 matmul per (h, kb) covering both
        # q-pairs whose window contains kb (halves LDWEIGHTS count -> PE
        # array duty stays above the HAM warm threshold). The e tile of a
        # k-tile is shared by the two q-pairs that read 256-col slices.
        with tc.tile_pool(name="qkps", bufs=5, space="PSUM") as qkps, \
             tc.tile_pool(name="expool", bufs=5) as expool, \
             tc.tile_pool(name="epool", bufs=10) as epool, \
             tc.tile_pool(name="rcp", bufs=4) as rcp, \
             tc.tile_pool(name="pvps", bufs=3, space="PSUM") as pvps:
            for h in range(HL):
                j, ro = h // 2, (h % 2) * 64
                e_tiles = {}
                for kb in range(NT):
                    qps = [q for q in range(NQP) if 2 * q - 1 <= kb <= 2 * q + 2]
                    q0 = qps[0] * 256
                    width = 256 * len(qps)
                    ps_s = qkps.tile([128, width], F32, tag="s",
                                     name=f"s{h}_{kb}")
                    nc.tensor.matmul(
                        ps_s[:],
                        kT[j][ro:ro + 64, kb * 128:(kb + 1) * 128],
                        qT[j][ro:ro + 64, q0:q0 + width],
                        start=True, stop=True,
                    )
                    # in-band columns: |kb*128 + i - q0 - col| <= 128
                    lo = max(0, ((kb * 128 - q0 - 128) // 128) * 128)
                    hi = min(width, ((kb * 128 - q0 + 256 + 127) // 128) * 128)
                    bw = hi - lo
                    boff = kb * 128 - q0 - lo  # 0 -> band256, 128 -> band384
                    bsl = (bins_sb[:, 384:384 + bw] if boff == 0
                           else bins_sb[:, 0:bw])
                    ex = expool.tile([128, width], F32, tag="ex",
                                     name=f"ex{h}_{kb}")
                    nc.scalar.activation(ex[:, lo:hi], ps_s[:, lo:hi],
                                         EXP, scale=SCALE)
                    e = epool.tile([128, width], F32R, tag="e",
                                   name=f"e{h}_{kb}")
                    nc.vector.tensor_mul(e[:, lo:hi], ex[:, lo:hi], bsl)
                    # columns outside [lo, hi) are structurally zero and are
                    # skipped by restricted PV matmuls below
                    e_tiles[kb] = (e, q0, lo, hi)

                for qp in range(NQP):
                    kbs = [kb for kb in range(2 * qp - 1, 2 * qp + 3)
                           if 0 <= kb < NT]
                    ps_o = pvps.tile([65, 256], F32, tag="o", name=f"o{h}_{qp}")
                    for i, kb in enumerate(kbs):
                        e, q0_e, lo_e, hi_e = e_tiles[kb]
                        off = qp * 256 - q0_e
                        # clip the 256-col slice to the in-band columns
                        a = max(off, lo_e)
                        b = min(off + 256, hi_e)
                        nc.tensor.matmul(
                            ps_o[:, a - off:b - off],
                            v_all[:, kb * VW + h * 65: kb * VW + (h + 1) * 65],
                            e[:, a:b],
                            start=(i == 0), stop=(i == len(kbs) - 1),
                        )
                    # normalize: fast reciprocal of denominator row, GPSIMD
                    # broadcast to 64 partitions, one DVE multiply psum->sbuf
                    d_sb = rcp.tile([1, 256], F32, tag="d", name=f"d{h}_{qp}")
                    nc.scalar.copy(d_sb[:], ps_o[64:65, :])
                    rc = rcp.tile([1, 256], F32, tag="rc", name=f"rc{h}_{qp}")
                    nc.vector.reciprocal_approx_fast(rc[:], d_sb[:])
                    rb = rcp.tile([64, 256], F32, tag="rb", name=f"rb{h}_{qp}")
                    nc.gpsimd.partition_broadcast(rb[:], rc[:])
                    nc.vector.tensor_mul(
                        attnT[j][ro:ro + 64, qp * 256:(qp + 1) * 256],
                        ps_o[0:64, :], rb[:])

        # ---------------- phase 3: projection ----------------
        with tc.tile_pool(name="wppool", bufs=1) as wppool, \
             tc.tile_pool(name="yps", bufs=3, space="PSUM") as yps, \
             tc.tile_pool(name="yp", bufs=3) as yp:
            wp_sb = []
            for cl in range(4):
                w_cl = wppool.tile([128, 1024], F32R, tag=f"wp{cl}", name=f"wp{cl}")
                nc.sync.dma_start(w_cl[:], wp_d[cl])
                wp_sb.append(w_cl)
            for t4 in range(4):
                for o in range(8):
                    ps_y = yps.tile([128, 512], F32, tag="y", name=f"y{t4}_{o}")
                    for cl in range(4):
                        nc.tensor.matmul(
                            ps_y[:],
                            wp_sb[cl][:, o * 128:(o + 1) * 128],
                            attnT[cl][:, t4 * 512:(t4 + 1) * 512],
                            start=(cl == 0), stop=(cl == 3),
                        )
                    y_o = yp.tile([128, 512], F32, tag="yo", name=f"yo{t4}_{o}")
                    nc.scalar.copy(y_o[:], ps_y[:])
                    nc.sync.dma_start(
                        yt_d[o * 128:(o + 1) * 128, t4 * 512:(t4 + 1) * 512],
                        y_o[:])

    nc.compile()
    return nc


def get_program():
    global _PROGRAM
    if _PROGRAM is None:
        _PROGRAM = build_program()
    return _PROGRAM


def _consts():
    # binary allow-masks in scoresT layout [k row i, q col]:
    # band384[i, c] = 1 iff |128 + i - c| <= 128 (i.e. i <= c <= i + 256)
    # band256[i, c] = 1 iff |i - c| <= 128
    i = np.arange(128)[:, None]
    c384 = np.arange(384)[None, :]
    c256 = np.arange(256)[None, :]
    band384 = (np.abs(128 + i - c384) <= 128).astype(np.float32)
    band256 = (np.abs(i - c256) <= 128).astype(np.float32)
    return {
        "ident": np.eye(128, dtype=np.float32),
        "ones": np.ones((128, 8), dtype=np.float32),
        "bins": np.concatenate([band384, band256], axis=1),
    }


def prepare_in_maps(x, w_qkv, w_proj):
    x = np.asarray(x, dtype=np.float32)
    w_qkv = np.asarray(w_qkv, dtype=np.float32)
    w_proj = np.asarray(w_proj, dtype=np.float32)
    consts = _consts()

    # xc[t, p, c*128+j] = x[b, t*128+j, c*128+p]  (contiguous 4KB partition rows)
    xcs = []
    for b in range(B):
        xc = x[b].reshape(NT, 128, NCT, 128).transpose(0, 3, 2, 1)
        xcs.append(round_f32r(np.ascontiguousarray(xc).reshape(NT, 128, 1024)))

    wqkvs, wps = [], []
    for hg in range(2):
        cols = []
        for part in range(3):
            sel = w_qkv[part * C + hg * 512: part * C + (hg + 1) * 512, :]  # [512,1024]
            cols.append(sel.T)                                   # [1024, 512]
        big = np.concatenate(cols, axis=1)                       # [1024, 1536]
        wqkvs.append(round_f32r(np.ascontiguousarray(big.reshape(NCT, 128, 1536))))
        wp = w_proj[:, hg * 512:(hg + 1) * 512].T.reshape(4, 128, 1024)
        wps.append(round_f32r(np.ascontiguousarray(wp)))

    in_maps = []
    for b in range(B):
        for hg in range(2):
            m = {"xc": xcs[b], "wqkv": wqkvs[hg], "wp": wps[hg]}
            m.update(consts)
            in_maps.append(m)
    return in_maps


def kernel(x, w_qkv, w_proj):
    nc = get_program()
    in_maps = prepare_in_maps(x, w_qkv, w_proj)
    res = run_bass_kernel_spmd(nc, in_maps, core_ids=list(range(8)))

    y = np.empty((B, T, C), dtype=np.float32)
    for b in range(B):
        yt = res.results[b * 2]["yt"] + res.results[b * 2 + 1]["yt"]
        y[b] = yt.T
    return y
